# revision 53
# baseline (speedup 1.0000x reference)
# Bidirectional cross-attention (talking heads) on 8 trn2 cores.
#
# Sharding: core c -> batch c//2, query-row half c%2 (audio rows). Each core
# computes the full attention for its 512 query rows against all 1024 text rows.
#
# Per-core dataflow (all matmuls bf16, fp32 accumulate):
#   LN(audio), LN(text) in [row, d] layout -> PE-transpose -> z^T [d, row]
#   qk^T = (g*scale*W_qk)^T @ z_a^T        [inner, 512]
#   v^T  = (g*W_v)^T @ z_t^T               [inner, 1024];  v = transpose(v^T)
#   sim^T[j,i] per head via K=64 row-tiled matmul pairs (2 heads concurrent)
#   e = exp(sim^T); ZP += e (vector fold over j-tiles, off the critical path)
#   DMA partition-shuffle e -> PK[(jt,g) partitions, j-local, i] via DRAM
#   (SBUF DMA APs are partition-dim-first on both sides, so an SBUF->SBUF
#   shuffle would explode into per-partition descriptors; the DRAM-side AP
#   carries the whole shuffle as one dense pattern instead)
#   Z[g,i] via 16 tiny one-column indicator matmuls on ZP;  PK *= 1/Z
#   talking-heads: block-diag W -> one 128x128 stationary matmul over PK,
#   cast back IN-PLACE into PK (each c-slice is dead once its chunk is read)
#   DMA shuffle back via DRAM -> MXq [j-local partitions, jt, h-quarter, i]
#   out2^T[(h,d), i] = v-tiles^T @ MXq  (M=64 col-tiled head pairs, N=512)
#   out = out2^T^T @ W_out (+ b_out added on host)
import numpy as np
import ml_dtypes
from contextlib import ExitStack

import concourse.bass as bass
import concourse.tile as tile
from concourse import mybir
from concourse.bass_utils import run_bass_kernel_spmd

BF16 = mybir.dt.bfloat16
F32 = mybir.dt.float32
AF = mybir.ActivationFunctionType
OP = mybir.AluOpType

HEADS, DH, D = 16, 64, 1024
I, J = 512, 1024  # per-core audio (query) rows, text rows
EPS = 1e-5
N_CORES = 8


def _layernorm_to_zT(nc, pools, x_src, zT, col0, eps_tile, ident):
    """DMA a [128, D] row-tile, layernorm core (no affine), transpose into
    zT[:, dt, col0:col0+128] (bf16, feature dim on partitions)."""
    xpool, stats, zbpool, tps = pools
    x = xpool.tile([128, D], F32)
    nc.sync.dma_start(out=x, in_=x_src)
    st = stats.tile([128, 2, 6], F32, tag="st")
    nc.vector.bn_stats(out=st[:, 0, :], in_=x[:, 0:512])
    nc.vector.bn_stats(out=st[:, 1, :], in_=x[:, 512:1024])
    mv = stats.tile([128, 2], F32, tag="mv")
    nc.vector.bn_aggr(out=mv, in_=st)
    rstd = stats.tile([128, 1], F32, tag="rstd")
    # rstd = 1/sqrt(var + eps)
    nc.scalar.activation(out=rstd, in_=mv[:, 1:2], func=AF.Sqrt, bias=eps_tile,
                         scale=1.0)
    nc.vector.reciprocal(out=rstd, in_=rstd)
    zb = zbpool.tile([128, D], BF16)
    nc.vector.tensor_scalar(out=zb, in0=x, scalar1=mv[:, 0:1], scalar2=rstd,
                            op0=OP.subtract, op1=OP.mult)
    for dt_ in range(8):
        ps = tps.tile([128, 128], BF16)
        nc.tensor.transpose(ps, zb[:, dt_ * 128:(dt_ + 1) * 128], ident)
        nc.any.tensor_copy(out=zT[:, dt_, col0:col0 + 128], in_=ps)


def _legalize_dma_waits(nc):
    """This container's walrus only supports ONE sync-wait on dynamic DMA
    instructions (PSEUDO_DMA_DIRECT2D).  Tile attaches several.  Move the
    excess onto EventSemaphore instructions inserted just before each DMA on
    the same issuing engine (evsems hold up to 2 waits each)."""
    import bass_rust as br

    def cap_of(ins):
        return 2 if type(ins).__name__ == "InstEventSemaphore" else 1

    n_fixed = 0
    for f in nc.m.functions:
        for blk in f.blocks:
            il = blk.instructions
            if not any(getattr(i, "sync_info", None)
                       and len(i.sync_info.on_wait) > cap_of(i) for i in il):
                continue
            newlist = []
            for ins in il:
                si = getattr(ins, "sync_info", None)
                cap = cap_of(ins)
                if si is not None and len(si.on_wait) > cap:
                    waits = list(si.on_wait)
                    extra, keep = waits[:-cap], waits[-cap:]
                    for k in range(0, len(extra), 2):
                        ev = mybir.InstEventSemaphore(
                            name=f"{ins.name}-wev{k}", ins=[], outs=[])
                        ev.engine = ins.engine
                        ev.sync_info = br.SyncInfo(on_wait=extra[k:k + 2],
                                                   on_update=[])
                        newlist.append(ev)
                    si.on_wait = keep
                    n_fixed += 1
                newlist.append(ins)
            blk.instructions = newlist
    return n_fixed


def build_nc(legalize=True):
    nc = bass.Bass()
    audio = nc.declare_dram_parameter("audio", [I, D], F32, isOutput=False)
    text = nc.declare_dram_parameter("text", [J, D], F32, isOutput=False)
    w1 = nc.declare_dram_parameter("w1", [D, D], BF16, isOutput=False)
    w2 = nc.declare_dram_parameter("w2", [D, D], BF16, isOutput=False)
    wout = nc.declare_dram_parameter("wout", [D, D], BF16, isOutput=False)
    c1 = nc.declare_dram_parameter("c1", [128, 8], F32, isOutput=False)
    c2 = nc.declare_dram_parameter("c2", [128, 8], F32, isOutput=False)
    wbig = nc.declare_dram_parameter("wbig", [128, 128], BF16, isOutput=False)
    zind = nc.declare_dram_parameter("zind", [128, 16, 16], BF16, isOutput=False)
    ident = nc.declare_dram_parameter("ident", [128, 128], BF16, isOutput=False)
    out = nc.declare_dram_parameter("out", [I, D], F32, isOutput=True)

    with tile.TileContext(nc) as tc, ExitStack() as ctx:
        singles = ctx.enter_context(tc.tile_pool(name="singles", bufs=1))
        persist = ctx.enter_context(tc.tile_pool(name="persist", bufs=1))

        # --- resident constants/weights (x row-tiles load first on sync via
        # the LN helper; park the big weights on gpsimd so they don't delay
        # the first layernorms) ---
        IDENT = singles.tile([128, 128], BF16)
        nc.sync.dma_start(out=IDENT, in_=ident[:, :])
        WOSB = singles.tile([128, 8, D], BF16)
        WBIGSB = singles.tile([128, 128], BF16)
        nc.gpsimd.dma_start(out=WBIGSB, in_=wbig[:, :])
        ZINDSB = singles.tile([128, 16, 16], BF16)
        nc.gpsimd.dma_start(out=ZINDSB, in_=zind[:, :, :])
        C1SB = singles.tile([128, 8], F32)
        nc.gpsimd.dma_start(out=C1SB, in_=c1[:, :])
        C2SB = singles.tile([128, 8], F32)
        nc.gpsimd.dma_start(out=C2SB, in_=c2[:, :])
        eps_tile = singles.tile([128, 1], F32)
        nc.vector.memset(eps_tile, EPS)

        # --- persistent activations ---
        VN = persist.tile([128, 8, D], BF16)      # v: [j-part, j-tile, inner]
        OUT2T = persist.tile([128, 8, I], BF16)   # out2^T: [inner-part, tile, i]

        # qk^T/v^T and the staging/Z tensors; QKT+VT live only through the
        # sim phase (the scope frees their SBUF for the mix/av stages)
        zppool = ctx.enter_context(tc.tile_pool(name="zp", bufs=1))
        stgpool = ctx.enter_context(tc.tile_pool(name="stg", bufs=2, space="DRAM"))
        ZP = zppool.tile([128, HEADS, I], BF16)
        stg1 = stgpool.tile([128, 128, I], BF16, tag="stg1")
        dma_engs = [nc.sync, nc.scalar, nc.gpsimd]

        with tc.tile_pool(name="simlife", bufs=1) as simlife:
            QKT = simlife.tile([128, 8, I], BF16)  # qk^T: [d-part, tile, i]
            VT = simlife.tile([128, 8, J], BF16)   # v^T:  [d-part, tile, j]

            # ============= Phase A: LN + transposes ==========================
            ZAT = simlife.tile([128, 8, I], BF16)
            ZTT = simlife.tile([128, 8, J], BF16)
            W1SB = simlife.tile([128, 8, D], BF16)
            nc.scalar.dma_start(out=W1SB, in_=w1[:, :].rearrange("(t p) n -> p t n", p=128))
            W2SB = simlife.tile([128, 8, D], BF16)
            nc.scalar.dma_start(out=W2SB, in_=w2[:, :].rearrange("(t p) n -> p t n", p=128))
            with tc.tile_pool(name="xp", bufs=3) as xpool, \
                 tc.tile_pool(name="stats", bufs=4) as stats, \
                 tc.tile_pool(name="zb", bufs=3) as zbpool, \
                 tc.tile_pool(name="tps", bufs=2, space="PSUM") as tps, \
                 tc.tile_pool(name="et", bufs=6) as etpool, \
                 tc.tile_pool(name="pps", bufs=2, space="PSUM") as pps, \
                 tc.tile_pool(name="simps", bufs=2, space="PSUM") as simps:
                pools = (xpool, stats, zbpool, tps)
                for it in range(4):
                    _layernorm_to_zT(nc, pools, audio[it * 128:(it + 1) * 128, :],
                                     ZAT, it * 128, eps_tile, IDENT)
                # qk^T for all inner-tiles: overlaps the text layernorms
                for mt in range(8):
                    ps = pps.tile([128, I], F32, tag="proj")
                    for kt in range(8):
                        nc.tensor.matmul(ps, W1SB[:, kt, mt * 128:(mt + 1) * 128],
                                         ZAT[:, kt, :], start=(kt == 0),
                                         stop=(kt == 7))
                    nc.scalar.activation(out=QKT[:, mt, :], in_=ps,
                                         func=AF.Identity,
                                         bias=C1SB[:, mt:mt + 1], scale=1.0)
                for jt in range(8):
                    _layernorm_to_zT(nc, pools, text[jt * 128:(jt + 1) * 128, :],
                                     ZTT, jt * 128, eps_tile, IDENT)

                # W_out is only needed by the final projection; load it after
                # the phase A inputs so it doesn't compete for HBM at startup
                nc.gpsimd.dma_start(out=WOSB, in_=wout[:, :].rearrange("(t p) n -> p t n", p=128))

                # ===== v^T projection + sim^T/exp in mt-pair quarters =======
                # sim for heads [4*th, 4*th+4) needs only inner-tiles
                # {2*th, 2*th+1} of v^T, so each projection quarter is
                # emitted just before the sim quarter that consumes it: the
                # scalar-bound exp stream starts after one quarter of the
                # v projection and overlaps the rest.
                for th in range(4):
                    for mt in range(2 * th, 2 * th + 2):
                        for nh in range(2):
                            ps = pps.tile([128, 512], F32, tag="proj")
                            for kt in range(8):
                                nc.tensor.matmul(ps, W2SB[:, kt, mt * 128:(mt + 1) * 128],
                                                 ZTT[:, kt, nh * 512:(nh + 1) * 512],
                                                 start=(kt == 0), stop=(kt == 7))
                            nc.scalar.activation(out=VT[:, mt, nh * 512:(nh + 1) * 512],
                                                 in_=ps, func=AF.Identity,
                                                 bias=C2SB[:, mt:mt + 1], scale=1.0)
                    for jt in range(8):
                        et = etpool.tile([128, 4, I], BF16)
                        for tl in range(2):
                            t = th * 2 + tl
                            ps = simps.tile([128, 2, I], F32, tag="sim")
                            nc.tensor.matmul(ps[:, 0, :],
                                             VT[0:64, t, jt * 128:(jt + 1) * 128],
                                             QKT[0:64, t, :],
                                             skip_group_check=True)
                            nc.tensor.matmul(ps[:, 1, :],
                                             VT[64:128, t, jt * 128:(jt + 1) * 128],
                                             QKT[64:128, t, :],
                                             skip_group_check=True)
                            nc.scalar.activation(out=et[:, 2 * tl:2 * tl + 2, :],
                                                 in_=ps, func=AF.Exp)
                        # stg1 layout [p=(jt,g), c, i]; dest walks (c, g, i)
                        [nc.sync, nc.gpsimd][(jt + th) % 2].dma_start(
                            out=stg1.rearrange("p c i -> c p i")[:, jt * 16 + th * 4:jt * 16 + th * 4 + 4, :],
                            in_=et)
                        if jt == 0:
                            nc.vector.tensor_copy(
                                out=ZP[:, th * 4:(th + 1) * 4, :], in_=et)
                        else:
                            nc.vector.tensor_add(
                                out=ZP[:, th * 4:(th + 1) * 4, :],
                                in0=ZP[:, th * 4:(th + 1) * 4, :], in1=et)

            # v natural layout: transpose VT.  Emitted after the sim matmuls
            # so the PE does this while the pack readback streams
            with tc.tile_pool(name="tps2", bufs=2, space="PSUM") as tps2:
                for mt in range(8):
                    for jt in range(8):
                        ps = tps2.tile([128, 128], BF16)
                        nc.tensor.transpose(ps, VT[:, mt, jt * 128:(jt + 1) * 128],
                                            IDENT)
                        nc.any.tensor_copy(out=VN[:, jt, mt * 128:(mt + 1) * 128],
                                           in_=ps)

        # ================= Phase B: normalize/mix/av at full i ===============
        # c-quarters of the packed tensor flow rb -> normalize -> mix ->
        # cast-in-place -> stg2 independently; then h-quarters flow
        # rb -> attn@v.  All free dims carry the full i=512 so every DMA
        # chunk is 1KB-contiguous and av matmuls stream N=512.
        with tc.tile_pool(name="pk", bufs=2) as pkpool, \
             tc.tile_pool(name="mxh", bufs=2) as mxhpool, \
             tc.tile_pool(name="zr", bufs=1) as zrpool, \
             tc.tile_pool(name="ob", bufs=2) as obpool, \
             tc.tile_pool(name="zps", bufs=1, space="PSUM") as zpsp, \
             tc.tile_pool(name="mixps", bufs=3, space="PSUM") as mixps, \
             tc.tile_pool(name="avps", bufs=2, space="PSUM") as avps, \
             tc.tile_pool(name="fpps", bufs=2, space="PSUM") as fpps:
            # --- Z[g,i] via 16 one-column indicator matmuls on ZP ---
            zps = zpsp.tile([16, I], F32)
            for g in range(16):
                nc.tensor.matmul(zps, ZINDSB[:, g, :], ZP[:, g, :],
                                 start=(g == 0), stop=(g == 15))
            zsb = zrpool.tile([16, I], F32, tag="zsb")
            nc.vector.reciprocal(out=zsb, in_=zps)
            zrb = zrpool.tile([16, I], BF16, tag="zrb")
            nc.any.tensor_copy(out=zrb, in_=zsb)
            ZRPK = zrpool.tile([128, I], BF16, tag="zrpk")
            for s in range(8):
                nc.sync.dma_start(out=ZRPK[s * 16:(s + 1) * 16, :], in_=zrb)
            zb_ap = bass.AP(tensor=ZRPK.tensor, offset=ZRPK.offset,
                            ap=[list(ZRPK.ap[0]), [0, 16], list(ZRPK.ap[1])])

            stg2 = stgpool.tile([128, 128, I], BF16, tag="stg2")
            for cq in range(4):
                cq0 = cq * 32
                PKq = pkpool.tile([128, 32, I], BF16)
                for cs in range(2):
                    nc.sync.dma_start(
                        out=PKq[:, cs * 16:(cs + 1) * 16, :],
                        in_=stg1[:, cq0 + cs * 16:cq0 + cs * 16 + 16, :])
                for cc in range(2):
                    nc.vector.tensor_mul(
                        out=PKq[:, cc * 16:(cc + 1) * 16, :],
                        in0=PKq[:, cc * 16:(cc + 1) * 16, :], in1=zb_ap)
                for cc in range(32):
                    mps = mixps.tile([128, 1, I], F32)
                    nc.tensor.matmul(mps, WBIGSB, PKq[:, cc:cc + 1, :])
                    if cc % 3 != 0:
                        nc.scalar.activation(out=PKq[:, cc:cc + 1, :],
                                             in_=mps, func=AF.Identity)
                    else:
                        nc.vector.tensor_copy(out=PKq[:, cc:cc + 1, :],
                                              in_=mps)
                for cw in range(2):
                    c0 = cw * 16
                    # source walks (p, c, i); dest stg2[c][p][i]
                    dma_engs[(2 * cq + cw) % 3].dma_start(
                        out=stg2.rearrange("c p i -> p c i")[:, cq0 + c0:cq0 + c0 + 16, :],
                        in_=PKq[:, c0:c0 + 16, :])

            # --- attn @ v, a quarter of the heads at a time -> out2^T ---
            for qt in range(4):
                MXq = mxhpool.tile([128, 8, 4, I], BF16)
                for he in range(2):
                    nc.sync.dma_start(
                        out=MXq[:, :, he * 2:he * 2 + 2, :],
                        in_=stg2.rearrange("c (s h) i -> c s h i", h=HEADS)[:, :, qt * 4 + he * 2:qt * 4 + he * 2 + 2, :])
                for t in range(2 * qt, 2 * qt + 2):
                    hl = 2 * t - 4 * qt
                    aps = avps.tile([128, I], F32)
                    for jt in range(8):
                        nc.tensor.matmul(aps[0:64, :],
                                         VN[:, jt, (2 * t) * 64:(2 * t + 1) * 64],
                                         MXq[:, jt, hl, :],
                                         start=(jt == 0), stop=(jt == 7),
                                         skip_group_check=True)
                        nc.tensor.matmul(aps[64:128, :],
                                         VN[:, jt, (2 * t + 1) * 64:(2 * t + 2) * 64],
                                         MXq[:, jt, hl + 1, :],
                                         start=(jt == 0), stop=(jt == 7),
                                         skip_group_check=True)
                    nc.scalar.activation(out=OUT2T[:, t, :],
                                         in_=aps, func=AF.Identity)

            # --- final projection ---
            for ic in range(4):
                r0 = ic * 128
                for nh in range(2):
                    fps = fpps.tile([128, 512], F32)
                    for kt in range(8):
                        nc.tensor.matmul(fps, OUT2T[:, kt, r0:r0 + 128],
                                         WOSB[:, kt, nh * 512:(nh + 1) * 512],
                                         start=(kt == 0), stop=(kt == 7))
                    ob = obpool.tile([128, 512], F32)
                    nc.scalar.activation(out=ob, in_=fps, func=AF.Identity)
                    nc.sync.dma_start(
                        out=out[r0:r0 + 128, nh * 512:(nh + 1) * 512],
                        in_=ob)
    if legalize:
        _legalize_dma_waits(nc)
    return nc


def _host_prep(text, audio, g_text, b_text, g_audio, b_audio, W_qk, W_v, W_out,
               b_out, W_th):
    bf16 = ml_dtypes.bfloat16
    scale = DH ** -0.5
    w1 = (g_audio[:, None] * W_qk * scale).astype(bf16)
    c1 = (scale * (b_audio @ W_qk)).astype(np.float32)
    w2 = (g_text[:, None] * W_v).astype(bf16)
    c2 = (b_text @ W_v).astype(np.float32)
    wout = W_out.astype(bf16)
    wbig = np.zeros((128, 128), np.float32)
    for s in range(8):
        wbig[s * 16:(s + 1) * 16, s * 16:(s + 1) * 16] = W_th.T
    wbig = wbig.astype(bf16)
    # zind[:, g, m] = 1 if m == g: one-column indicators for the Z matmuls
    zind = np.zeros((128, 16, 16), np.float32)
    for g in range(16):
        zind[:, g, g] = 1.0
    zind = zind.astype(bf16)
    ident = np.eye(128, dtype=np.float32).astype(bf16)
    # pack [1024] -> [128, 8] with c[p, t] = vec[t*128 + p]
    c1p = np.ascontiguousarray(c1.reshape(8, 128).T)
    c2p = np.ascontiguousarray(c2.reshape(8, 128).T)
    shared = dict(w1=w1, w2=w2, wout=wout, c1=c1p, c2=c2p, wbig=wbig,
                  zind=zind, ident=ident)
    in_maps = []
    for core in range(N_CORES):
        b, half = core // 2, core % 2
        in_maps.append(dict(
            audio=np.ascontiguousarray(audio[b, half * I:(half + 1) * I, :],
                                       dtype=np.float32),
            text=np.ascontiguousarray(text[b], dtype=np.float32),
            **shared))
    return in_maps


_NC = None


def _get_nc():
    global _NC
    if _NC is None:
        _NC = build_nc()
    return _NC


def kernel(text, audio, g_text, b_text, g_audio, b_audio, W_qk, W_v, W_out,
           b_out, W_th, _trace=False):
    text = np.asarray(text, np.float32)
    audio = np.asarray(audio, np.float32)
    in_maps = _host_prep(np.asarray(text, np.float32),
                         np.asarray(audio, np.float32),
                         np.asarray(g_text, np.float32),
                         np.asarray(b_text, np.float32),
                         np.asarray(g_audio, np.float32),
                         np.asarray(b_audio, np.float32),
                         np.asarray(W_qk, np.float32),
                         np.asarray(W_v, np.float32),
                         np.asarray(W_out, np.float32),
                         np.asarray(b_out, np.float32),
                         np.asarray(W_th, np.float32))
    nc = _get_nc()
    res = run_bass_kernel_spmd(nc, in_maps, list(range(N_CORES)), trace=_trace)
    b_ = audio.shape[0]
    full = np.empty((b_, 2 * I, D), np.float32)
    for core in range(N_CORES):
        b, half = core // 2, core % 2
        full[b, half * I:(half + 1) * I, :] = res.results[core]["out"]
    full += np.asarray(b_out, np.float32)[None, None, :]
    if _trace:
        return full, res
    return full
